# revision 1
# baseline (speedup 1.0000x reference)
"""Bahdanau-attention kernel for Trainium2, data-parallel over 8 NeuronCores.

Per core (B_local=8, T=1024, H=1024), for each batch b:
  encT[o,t] = sum_h W_enc[o,h] * x[t,h]        (PE, f32r, x PE-transposed)
  energyT   = tanh(encT + (W_dec h + b_dec + b_enc)[o])   (ScalarE, fused bias)
  scores[t] = sum_o w_score[o] * energyT[o,t]  (PE, w_score stationary)
  weights   = softmax(scores)                  (DVE/ScalarE, 1 partition)
  context   = sum_t weights[t] * x[t,:]        (PE, natural-layout x)
b_score is dropped: softmax is shift-invariant so it cancels in both outputs.
"""

import sys

if "/opt/trn_rl_repo" not in sys.path:
    sys.path.insert(0, "/opt/trn_rl_repo")

import numpy as np

B, T, H = 64, 1024, 1024
NCORES = 8
BL = B // NCORES  # batches per core
P = 128  # partitions
NT = T // P  # t tiles
NH = H // P  # h chunks
NO = H // P  # o tiles
F = 512  # matmul free-dim slice (one PSUM bank of f32)
NS = T // F  # free-dim halves

_CACHE = {}
LAST_RESULT = None


def build(bl=BL):
    import concourse.tile as tile
    from concourse import bacc, mybir
    from concourse.masks import make_identity

    f32 = mybir.dt.float32
    f32r = mybir.dt.float32r
    AF = mybir.ActivationFunctionType
    AX = mybir.AxisListType

    nc = bacc.Bacc("TRN2", target_bir_lowering=False, debug=False, num_devices=NCORES)
    x_d = nc.declare_dram_parameter("spatial_feats", [bl, T, H], f32r, isOutput=False)
    hs_d = nc.declare_dram_parameter("hidden_state", [bl, H], f32r, isOutput=False)
    we_d = nc.declare_dram_parameter("W_enc", [H, H], f32r, isOutput=False)
    be_d = nc.declare_dram_parameter("b_enc", [H], f32, isOutput=False)
    wd_d = nc.declare_dram_parameter("W_dec", [H, H], f32r, isOutput=False)
    bd_d = nc.declare_dram_parameter("b_dec", [H], f32, isOutput=False)
    ws_d = nc.declare_dram_parameter("w_score", [H], f32r, isOutput=False)
    scr_d = nc.declare_dram_parameter("sc_scratch", [bl, T], f32, isOutput=True)
    ctx_d = nc.declare_dram_parameter("out_ctx", [bl, H], f32, isOutput=True)
    wout_d = nc.declare_dram_parameter("out_w", [bl, T], f32, isOutput=True)

    with tile.TileContext(nc) as tc:
        with (
            tc.tile_pool(name="const", bufs=1) as const,
            tc.tile_pool(name="wt", bufs=NH) as wt_pool,
            tc.tile_pool(name="xnat", bufs=2 * NT + 4) as xnat_pool,
            tc.tile_pool(name="xT", bufs=NH) as xT_pool,
            tc.tile_pool(name="eT", bufs=6) as eT_pool,
            tc.tile_pool(name="rows", bufs=1) as rows,
            tc.tile_pool(name="wdtmp", bufs=4) as wdtmp,
            tc.tile_pool(name="urows", bufs=2) as urows,
            tc.tile_pool(name="small", bufs=12) as small,
            tc.tile_pool(name="mmps", bufs=2, space="PSUM") as mm_ps,
            tc.tile_pool(name="trps", bufs=3, space="PSUM") as tr_ps,
            tc.tile_pool(name="scps", bufs=1, space="PSUM") as sc_ps,
            tc.tile_pool(name="ctxps", bufs=2, space="PSUM") as ctx_ps,
        ):
            # ---- constants ----
            ident_f = const.tile([P, P], f32, tag="identf")
            make_identity(nc, ident_f[:])
            ident = const.tile([P, P], f32r, tag="ident")
            nc.vector.tensor_copy(ident[:], ident_f[:])
            ones8_f = const.tile([1, NT], f32, tag="ones8f")
            nc.gpsimd.memset(ones8_f[:], 1.0)
            ones8 = const.tile([1, NT], f32r, tag="ones8")
            nc.vector.tensor_copy(ones8[:], ones8_f[:])

            def transpose_block(nat_tiles, dst_tile, j):
                """dst_tile <- transpose of column-block j of nat tiles (f32r)."""
                n_rows = len(nat_tiles)
                for half in range((n_rows + 3) // 4):
                    q_cnt = min(4, n_rows - half * 4)
                    ps = tr_ps.tile([P, F], f32r, tag="trps")
                    for q in range(q_cnt):
                        tt = half * 4 + q
                        nc.tensor.transpose(
                            ps[:, q * P : (q + 1) * P],
                            nat_tiles[tt][:, j * P : (j + 1) * P],
                            ident[:],
                        )
                    nc.vector.tensor_copy(
                        dst_tile[:, half * F : half * F + q_cnt * P],
                        ps[:, 0 : q_cnt * P],
                    )

            def transpose_tile(nat, dsts, tt):
                """Column tt of all NH dst tiles <- transpose of one nat tile.
                DMA-paced: only needs this one nat tile resident."""
                for half in range(2):
                    ps = tr_ps.tile([P, F], f32r, tag="trps")
                    for q in range(4):
                        j = half * 4 + q
                        nc.tensor.transpose(
                            ps[:, q * P : (q + 1) * P], nat[:, j * P : (j + 1) * P],
                            ident[:],
                        )
                    for q in range(4):
                        j = half * 4 + q
                        nc.vector.tensor_copy(
                            dsts[j][:, tt * P : (tt + 1) * P], ps[:, q * P : (q + 1) * P]
                        )

            # ---- x(0): DMA first, transpose per-tile as DMA lands ----
            xnat0 = []
            _engs = [nc.sync, nc.scalar, nc.gpsimd]
            for tt in range(NT):
                t = xnat_pool.tile([P, H], f32r, tag="xnat")
                _engs[tt % 3].dma_start(t[:], x_d[0, tt * P : (tt + 1) * P, :])
                xnat0.append(t)
            be_sb = const.tile([P, NH], f32, tag="be")
            nc.gpsimd.dma_start(be_sb[:], be_d[:].rearrange("(c p) -> p c", p=P))
            bd_sb = const.tile([P, NH], f32, tag="bd")
            nc.gpsimd.dma_start(bd_sb[:], bd_d[:].rearrange("(c p) -> p c", p=P))
            bsum = const.tile([P, NH], f32, tag="bsum")
            nc.vector.tensor_add(bsum[:], be_sb[:], bd_sb[:])
            ws_sb = const.tile([P, NH], f32r, tag="ws")
            nc.gpsimd.dma_start(ws_sb[:], ws_d[:].rearrange("(c p) -> p c", p=P))
            hsT = const.tile([P, NH * bl], f32r, tag="hsT")
            for c in range(NH):
                nc.gpsimd.dma_start(
                    hsT[:, c * bl : (c + 1) * bl],
                    hs_d[:, c * P : (c + 1) * P].rearrange("b p -> p b"),
                )

            xT0 = [xT_pool.tile([P, T], f32r, tag="xT", name=f"xT0_{j}") for j in range(NH)]
            for tt in range(NT):
                transpose_tile(xnat0[tt], xT0, tt)

            # ---- per o_tile: W_dec bias, W_enc transposes, and batch-0 enc ----
            # Interleaved so PE has dense real work as soon as DMA lands (warms
            # the HAM clock gate early and hides W processing under enc).
            bias_all = const.tile([P, NO * bl], f32, tag="bias_all")
            wT = [wt_pool.tile([P, H], f32r, tag="wt", name=f"wT_{j}") for j in range(NH)]
            eT0 = {}
            for ot in range(NO):
                wdnat = xnat_pool.tile([P, H], f32r, tag="xnat")
                _engs[(2 * ot) % 3].dma_start(
                    wdnat[:, 0 : H // 2], wd_d[ot * P : (ot + 1) * P, 0 : H // 2]
                )
                _engs[(2 * ot + 1) % 3].dma_start(
                    wdnat[:, H // 2 :], wd_d[ot * P : (ot + 1) * P, H // 2 :]
                )
                wdT = []
                for half in range(2):
                    ps = tr_ps.tile([P, F], f32r, tag="trps")
                    for q in range(4):
                        j = half * 4 + q
                        nc.tensor.transpose(
                            ps[:, q * P : (q + 1) * P], wdnat[:, j * P : (j + 1) * P],
                            ident[:],
                        )
                    w4 = wdtmp.tile([P, F], f32r, tag="wdT")
                    nc.vector.tensor_copy(w4[:], ps[:])
                    wdT.append(w4)
                psd = mm_ps.tile([P, F], f32, tag="mmps")
                for h in range(NH):
                    nc.tensor.matmul(
                        psd[:, 0:bl],
                        wdT[h // 4][:, (h % 4) * P : (h % 4 + 1) * P],
                        hsT[:, h * bl : (h + 1) * bl],
                        start=(h == 0),
                        stop=(h == NH - 1),
                    )
                nc.vector.tensor_scalar_add(
                    bias_all[:, ot * bl : (ot + 1) * bl], psd[:, 0:bl],
                    bsum[:, ot : ot + 1],
                )
                wenat = xnat_pool.tile([P, H], f32r, tag="xnat")
                _engs[(2 * ot + 2) % 3].dma_start(
                    wenat[:, 0 : H // 2], we_d[ot * P : (ot + 1) * P, 0 : H // 2]
                )
                _engs[(2 * ot) % 3].dma_start(
                    wenat[:, H // 2 :], we_d[ot * P : (ot + 1) * P, H // 2 :]
                )
                transpose_tile(wenat, wT, ot)
                for half in range(NS):
                    ps = mm_ps.tile([P, F], f32, tag="mmps")
                    for h in range(NH):
                        nc.tensor.matmul(
                            ps[:],
                            wT[h][:, ot * P : (ot + 1) * P],
                            xT0[h][:, half * F : (half + 1) * F],
                            start=(h == 0),
                            stop=(h == NH - 1),
                        )
                    e = eT_pool.tile([P, F], f32r, tag="eT")
                    nc.scalar.activation(
                        e[:], ps[:], AF.Tanh,
                        bias=bias_all[:, ot * bl : ot * bl + 1], scale=1.0,
                    )
                    eT0[(ot, half)] = e
                if ot == 0:
                    pss0_h = {
                        0: sc_ps.tile([1, F], f32, tag="scps", name="pss0_h0"),
                        1: ctx_ps.tile([1, F], f32, tag="ctxps", name="pss0_h1"),
                    }
                else:
                    for half in range(NS):
                        nc.tensor.matmul(
                            pss0_h[half][:],
                            ws_sb[:, ot - 1 : ot],
                            eT0[(ot - 1, half)][:],
                            start=(ot - 1 == 0),
                            stop=False,
                        )

            # ---- main per-batch pipeline ----
            prev = None
            xnat, xT = xnat0, xT0
            for it in range(bl + 1):
                # stage 1: DMA + transpose x for batch `it` (batch 0 in preamble)
                if 0 < it < bl:
                    xnat = []
                    for tt in range(NT):
                        t = xnat_pool.tile([P, H], f32r, tag="xnat")
                        dma_eng = nc.sync if tt % 2 == 0 else nc.gpsimd
                        dma_eng.dma_start(t[:], x_d[it, tt * P : (tt + 1) * P, :])
                        xnat.append(t)
                    xT = []
                    for j in range(NH):
                        xj = xT_pool.tile([P, T], f32r, tag="xT")
                        transpose_block(xnat, xj, j)
                        xT.append(xj)

                # stage 2: finish batch it-1 (exp-weight transpose + context)
                if it >= 1:
                    b, st = it - 1, prev
                    u_col = st["uT"]
                    ctx_row = rows.tile([1, H], f32, tag="ctxrow")
                    for half in range(NS):
                        pc = ctx_ps.tile([1, F], f32, tag="ctxps")
                        for c in range(NT):
                            nc.tensor.matmul(
                                pc[:],
                                u_col[:, c : c + 1],
                                st["xnat"][c][:, half * F : (half + 1) * F],
                                start=(c == 0),
                                stop=(c == NT - 1),
                            )
                        nc.vector.tensor_scalar_mul(
                            ctx_row[0:1, half * F : (half + 1) * F],
                            pc[0:1, :],
                            st["rz"][0:1, 0:1],
                        )
                    nc.sync.dma_start(ctx_d[b : b + 1, :], ctx_row[:])
                    w_row = rows.tile([1, T], f32, tag="wrow")
                    nc.vector.tensor_scalar_mul(
                        w_row[:], st["u_row"][:], st["rz"][0:1, 0:1]
                    )
                    nc.sync.dma_start(wout_d[b : b + 1, :], w_row[:])

                # stage 3: enc + scores + softmax for batch `it`
                # (scores run one o_tile behind enc, so eT tiles free early
                # and the softmax tail starts right after the last enc group)
                if it < bl:
                    sc_row = rows.tile([1, T], f32, tag="scrow")
                    if it == 0:
                        eT, pss_h = eT0, pss0_h
                    else:
                        eT = {}
                        pss_h = {}
                        for o in range(NO):
                            for half in range(NS):
                                ps = mm_ps.tile([P, F], f32, tag="mmps")
                                for h in range(NH):
                                    nc.tensor.matmul(
                                        ps[:],
                                        wT[h][:, o * P : (o + 1) * P],
                                        xT[h][:, half * F : (half + 1) * F],
                                        start=(h == 0),
                                        stop=(h == NH - 1),
                                    )
                                e = eT_pool.tile([P, F], f32r, tag="eT")
                                nc.scalar.activation(
                                    e[:],
                                    ps[:],
                                    AF.Tanh,
                                    bias=bias_all[:, o * bl + it : o * bl + it + 1],
                                    scale=1.0,
                                )
                                eT[(o, half)] = e
                            if o == 0:
                                pss_h[0] = sc_ps.tile([1, F], f32, tag="scps", name="pss_h0")
                                pss_h[1] = ctx_ps.tile([1, F], f32, tag="ctxps", name="pss_h1")
                            else:
                                for half in range(NS):
                                    nc.tensor.matmul(
                                        pss_h[half][:],
                                        ws_sb[:, o - 1 : o],
                                        eT[(o - 1, half)][:],
                                        start=(o - 1 == 0),
                                        stop=False,
                                    )
                    for half in range(NS):
                        nc.tensor.matmul(
                            pss_h[half][:],
                            ws_sb[:, NO - 1 : NO],
                            eT[(NO - 1, half)][:],
                            start=False,
                            stop=True,
                        )
                        nc.vector.tensor_copy(
                            sc_row[0:1, half * F : (half + 1) * F], pss_h[half][0:1, :]
                        )
                    uT = small.tile([P, NT], f32r, tag="uT")
                    if it == bl - 1:
                        # tail-latency path: columnize exp-weights on PE via
                        # K=1 matmuls against a ones row (no DRAM round-trip)
                        u_rowr = urows.tile([1, T], f32r, tag="urowr")
                        nc.scalar.activation(
                            u_rowr[:], sc_row[:], AF.Exp, bias=0.0, scale=1.0
                        )
                        psw = tr_ps.tile([P, NT * NT], f32, tag="trps")
                        for c in range(NT):
                            nc.tensor.matmul(
                                psw[:, c * NT : (c + 1) * NT],
                                u_rowr[0:1, c * P : (c + 1) * P],
                                ones8[:],
                                start=True,
                                stop=True,
                            )
                        nc.vector.tensor_copy(uT[:], psw[:, 0 : NT * NT : NT])
                    else:
                        nc.sync.dma_start(scr_d[it : it + 1, :], sc_row[0:1, :])
                        scT = small.tile([P, NT], f32, tag="scT")
                        nc.sync.dma_start(
                            scT[:], scr_d[it, :].rearrange("(c p) -> p c", p=P)
                        )
                        nc.scalar.activation(
                            uT[:], scT[:], AF.Exp, bias=0.0, scale=1.0
                        )
                    u_row = urows.tile([1, T], f32, tag="urow")
                    ssum = small.tile([1, 1], f32, tag="ssum")
                    nc.scalar.activation(
                        u_row[:],
                        sc_row[:],
                        AF.Exp,
                        bias=0.0,
                        scale=1.0,
                        accum_out=ssum[:],
                    )
                    rz = small.tile([1, 1], f32, tag="rz")
                    nc.vector.reciprocal(rz[:], ssum[:])
                    prev = {"xnat": xnat, "u_row": u_row, "uT": uT, "rz": rz}

    nc.compile()
    return nc


def _get_nc(bl=BL):
    if bl not in _CACHE:
        _CACHE[bl] = build(bl)
    return _CACHE[bl]


def kernel(**inputs):
    from concourse.bass_utils import run_bass_kernel_spmd

    x = np.ascontiguousarray(np.asarray(inputs["spatial_feats"], dtype=np.float32))
    hs = np.ascontiguousarray(np.asarray(inputs["hidden_state"], dtype=np.float32))
    shared = {
        k: np.ascontiguousarray(np.asarray(inputs[k], dtype=np.float32))
        for k in ("W_enc", "b_enc", "W_dec", "b_dec", "w_score")
    }
    nc = _get_nc()
    in_maps = []
    for i in range(NCORES):
        m = {
            "spatial_feats": x[i * BL : (i + 1) * BL],
            "hidden_state": hs[i * BL : (i + 1) * BL],
        }
        m.update(shared)
        in_maps.append(m)
    res = run_bass_kernel_spmd(nc, in_maps, core_ids=list(range(NCORES)))
    global LAST_RESULT
    LAST_RESULT = res
    ctx = np.concatenate([res.results[i]["out_ctx"] for i in range(NCORES)], axis=0)
    w = np.concatenate([res.results[i]["out_w"] for i in range(NCORES)], axis=0)
    return (ctx, w)



# revision 6
# speedup vs baseline: 1.4019x; 1.4019x over previous
"""Bahdanau-attention kernel for Trainium2, data-parallel over 8 NeuronCores.

Per core (B_local=8, T=1024, H=1024), per batch b:
  encT[o,t] = sum_h W_enc[o,h] * x[t,h]   (PE, fp8 e4m3 DoubleRow: each call
      contracts 2 K-tiles of 128. K_PAIR chunks ride as plain-quantized
      pairs; the rest ride as (x_hi, 16*(x-x_hi)) against (256*W, 16*W),
      making x exact to 2nd order. All W scaled by 256 -> tanh scale 2^-8.)
  energyT   = tanh(encT*2^-8 + (W_dec h + b_dec + b_enc)[o])  (ScalarE)
  scores[t] = sum_o w_score[o] * energyT[o,t]   (PE, bf16)
  weights   = softmax(scores)   (exp on ScalarE straight from PSUM,
      weight columnization via K=1 matmuls against a ones row)
  context   = sum_t weights[t] * x[t,:]         (PE, bf16 natural-layout x)
b_score dropped: softmax shift-invariance cancels it in both outputs.
All transposes are done host-side (xT fp8 pairs, W_encT fp8, W_decT/hsT
bf16, x natural bf16) -- no PE transposes in steady state.
"""

import sys

if "/opt/trn_rl_repo" not in sys.path:
    sys.path.insert(0, "/opt/trn_rl_repo")

import numpy as np

B, T, H = 64, 1024, 1024
NCORES = 8
BL = B // NCORES
P = 128
NT = T // P  # t chunks
NH = H // P  # h chunks
NO = H // P  # o blocks
F = 512      # PSUM bank free size (f32)
NS = T // F
Q = 256      # DoubleRow moving free (rhs free = 2*Q = 512)
K_PAIR = 4   # h-chunks in plain pair mode; rest in split (x-exact) mode
NCALL = K_PAIR // 2 + (NH - K_PAIR)

_CACHE = {}
LAST_RESULT = None


def build(bl=BL):
    import concourse.tile as tile
    from concourse import bacc, mybir
    from concourse.masks import make_identity

    f32 = mybir.dt.float32
    f8 = mybir.dt.float8e4
    bf = mybir.dt.bfloat16
    AF = mybir.ActivationFunctionType
    DR = mybir.MatmulPerfMode.DoubleRow

    nc = bacc.Bacc("TRN2", target_bir_lowering=False, debug=False, num_devices=NCORES)
    xp_d = nc.declare_dram_parameter("x_pairs", [bl, P, NCALL, 2, T], f8, isOutput=False)
    xn_d = nc.declare_dram_parameter("x_nat", [bl, P, NT, H], bf, isOutput=False)
    wp_d = nc.declare_dram_parameter("w_pairs", [NO, P, NCALL, 2, P], f8, isOutput=False)
    wdT_d = nc.declare_dram_parameter("wdT", [NH, P, H], bf, isOutput=False)
    hsT_d = nc.declare_dram_parameter("hsT", [P, NH * bl], bf, isOutput=False)
    ws_d = nc.declare_dram_parameter("ws_col", [P, NH], bf, isOutput=False)
    be_d = nc.declare_dram_parameter("be_col", [P, NH], f32, isOutput=False)
    bd_d = nc.declare_dram_parameter("bd_col", [P, NH], f32, isOutput=False)
    ctx_d = nc.declare_dram_parameter("out_ctx", [bl, H], f32, isOutput=True)
    wout_d = nc.declare_dram_parameter("out_w", [bl, T], f32, isOutput=True)

    with tile.TileContext(nc) as tc:
        with (
            tc.tile_pool(name="const", bufs=1) as const,
            tc.tile_pool(name="wenc", bufs=NO) as wenc_pool,
            tc.tile_pool(name="wdec", bufs=NH) as wdec_pool,
            tc.tile_pool(name="xp", bufs=2) as xp_pool,
            tc.tile_pool(name="xn", bufs=3) as xn_pool,
            tc.tile_pool(name="eT", bufs=6) as eT_pool,
            tc.tile_pool(name="rows", bufs=2) as rows,
            tc.tile_pool(name="small", bufs=2) as small,
            tc.tile_pool(name="mmps", bufs=4, space="PSUM") as mm_ps,
            tc.tile_pool(name="scps", bufs=2, space="PSUM") as sc_ps,
            tc.tile_pool(name="ctxps", bufs=2, space="PSUM") as ctx_ps,
        ):
            # ---- constants / weights DMA ----
            hsT = const.tile([P, NH * bl], bf, tag="hsT")
            nc.sync.dma_start(hsT[:], hsT_d[:, :])
            wdT = []
            for c in range(NH):
                t = wdec_pool.tile([P, H], bf, tag="wdT")
                eng = nc.gpsimd if c % 2 == 0 else nc.scalar
                eng.dma_start(t[:], wdT_d[c, :, :])
                wdT.append(t)
            ws_sb = const.tile([P, NH], bf, tag="ws")
            nc.sync.dma_start(ws_sb[:], ws_d[:, :])
            be_sb = const.tile([P, NH], f32, tag="be")
            nc.sync.dma_start(be_sb[:], be_d[:, :])
            bd_sb = const.tile([P, NH], f32, tag="bd")
            nc.sync.dma_start(bd_sb[:], bd_d[:, :])
            bsum = const.tile([P, NH], f32, tag="bsum")
            nc.vector.tensor_add(bsum[:], be_sb[:], bd_sb[:])

            ident_f = const.tile([P, P], f32, tag="identf")
            make_identity(nc, ident_f[:])
            ones_f = const.tile([1, NT], f32, tag="onesf")
            nc.gpsimd.memset(ones_f[:], 1.0)
            ones8 = const.tile([1, NT], bf, tag="ones8")
            nc.vector.tensor_copy(ones8[:], ones_f[:])

            wT = []
            for ob in range(NO):
                t = wenc_pool.tile([P, NCALL, 2, P], f8, tag="wenc")
                nc.sync.dma_start(t[:], wp_d[ob, :, :, :, :])
                wT.append(t)

            # ---- batch-0 x DMA ----
            def dma_x(it):
                xp_t = xp_pool.tile([P, NCALL, 2, T], f8, tag="xp")
                half = NCALL // 2
                nc.sync.dma_start(xp_t[:, 0:half, :, :], xp_d[it, :, 0:half, :, :])
                nc.gpsimd.dma_start(xp_t[:, half:, :, :], xp_d[it, :, half:, :, :])
                xn_t = xn_pool.tile([P, NT, H], bf, tag="xn")
                nc.gpsimd.dma_start(xn_t[:, 0:4, :], xn_d[it, :, 0:4, :])
                nc.scalar.dma_start(xn_t[:, 4:8, :], xn_d[it, :, 4:8, :])
                return xp_t, xn_t

            xp_cur, xn_cur = dma_x(0)

            # ---- dec bias: dec[b,o] = sum_h hs[b,h] Wd[o,h]; then transpose ----
            dec_sb = const.tile([bl, H], f32, tag="decsb")
            for half in range(NS):
                psd = mm_ps.tile([P, F], f32, tag="mmps")
                for c in range(NH):
                    nc.tensor.matmul(
                        psd[0:bl, :],
                        hsT[:, c * bl : (c + 1) * bl],
                        wdT[c][:, half * F : (half + 1) * F],
                        start=(c == 0),
                        stop=(c == NH - 1),
                    )
                nc.vector.tensor_copy(dec_sb[:, half * F : (half + 1) * F], psd[0:bl, :])
            bias_all = const.tile([P, NO * bl], f32, tag="bias_all")
            for ob in range(NO):
                psT = mm_ps.tile([P, F], f32, tag="mmps")
                nc.tensor.transpose(
                    psT[:, 0:bl], dec_sb[:, ob * P : (ob + 1) * P], ident_f[0:bl, 0:bl]
                )
                nc.vector.tensor_scalar_add(
                    bias_all[:, ob * bl : (ob + 1) * bl], psT[:, 0:bl],
                    bsum[:, ob : ob + 1],
                )

            # ---- helpers ----
            def enc_ob(it, ob, xp_t):
                """enc+tanh for one o-block; returns (eT_half0, eT_half1)."""
                psE = [mm_ps.tile([P, F], f32, tag="mmps", name=f"psE{ob}_{h}")
                       for h in range(NS)]
                for c in range(NCALL):
                    for half in range(NS):
                        nc.tensor.matmul(
                            psE[half][:],
                            wT[ob][:, c, :, :],
                            xp_t[:, c, :, half * F : (half + 1) * F],
                            start=(c == 0),
                            stop=(c == NCALL - 1),
                            perf_mode=DR,
                        )
                es = []
                for half in range(NS):
                    e = eT_pool.tile([P, F], bf, tag="eT")
                    nc.scalar.activation(
                        e[:], psE[half][:], AF.Tanh,
                        bias=bias_all[:, ob * bl + it : ob * bl + it + 1],
                        scale=1.0 / 256.0,
                    )
                    es.append(e)
                return es

            def score_ob(ob, ets, pss):
                for half in range(NS):
                    nc.tensor.matmul(
                        pss[half][:],
                        ws_sb[:, ob : ob + 1],
                        ets[ob][half][:],
                        start=(ob == 0),
                        stop=(ob == NO - 1),
                    )

            # ---- main pipeline ----
            prev = None
            for it in range(bl + 1):
                if 0 < it < bl:
                    xp_cur, xn_cur = dma_x(it)

                if it < bl:
                    # o-block 0 first: gives PE work while prev softmax drains
                    ets = [enc_ob(it, 0, xp_cur)]

                if it >= 1:
                    st = prev
                    # columnize exp-weights: uT[:, c] = u[c*128:(c+1)*128]
                    psw = mm_ps.tile([P, NT * NT], f32, tag="mmps")
                    for c in range(NT):
                        nc.tensor.matmul(
                            psw[:, c * NT : (c + 1) * NT],
                            st["u_rowr"][0:1, c * P : (c + 1) * P],
                            ones8[:],
                            start=(c == 0),
                            stop=(c == NT - 1),
                        )
                    uT = small.tile([P, NT], bf, tag="uT")
                    nc.vector.tensor_copy(uT[:], psw[:, 0 : NT * NT : NT])
                    ctx_row = rows.tile([1, H], f32, tag="ctxrow")
                    for half in range(NS):
                        pc = ctx_ps.tile([1, F], f32, tag="ctxps")
                        for c in range(NT):
                            nc.tensor.matmul(
                                pc[:],
                                uT[:, c : c + 1],
                                st["xn"][:, c, half * F : (half + 1) * F],
                                start=(c == 0),
                                stop=(c == NT - 1),
                            )
                        nc.vector.tensor_scalar_mul(
                            ctx_row[0:1, half * F : (half + 1) * F],
                            pc[0:1, :],
                            st["rz"][0:1, 0:1],
                        )
                    nc.sync.dma_start(ctx_d[it - 1 : it, :], ctx_row[:])
                    w_row = rows.tile([1, T], f32, tag="wrow")
                    nc.vector.tensor_scalar_mul(
                        w_row[:], st["u_row"][:], st["rz"][0:1, 0:1]
                    )
                    nc.sync.dma_start(wout_d[it - 1 : it, :], w_row[:])

                if it < bl:
                    pss = [sc_ps.tile([1, F], f32, tag="scps", name=f"pss{h}")
                           for h in range(NS)]
                    for ob in range(1, NO):
                        ets.append(enc_ob(it, ob, xp_cur))
                        score_ob(ob - 1, ets, pss)
                    score_ob(NO - 1, ets, pss)
                    # softmax pieces: exp straight from PSUM
                    u_rowr = rows.tile([1, T], bf, tag="urowr")
                    u_row = rows.tile([1, T], f32, tag="urow")
                    ssum = small.tile([1, NS], f32, tag="ssum")
                    for half in range(NS):
                        nc.scalar.activation(
                            u_rowr[0:1, half * F : (half + 1) * F],
                            pss[half][0:1, :], AF.Exp, bias=0.0, scale=1.0,
                        )
                    for half in range(NS):
                        nc.scalar.activation(
                            u_row[0:1, half * F : (half + 1) * F],
                            pss[half][0:1, :], AF.Exp, bias=0.0, scale=1.0,
                            accum_out=ssum[0:1, half : half + 1],
                        )
                    stot = small.tile([1, 1], f32, tag="stot")
                    nc.vector.tensor_add(stot[:], ssum[0:1, 0:1], ssum[0:1, 1:2])
                    rz = small.tile([1, 1], f32, tag="rz")
                    nc.vector.reciprocal(rz[:], stot[:])
                    prev = {"u_rowr": u_rowr, "u_row": u_row, "rz": rz, "xn": xn_cur}

    nc.compile()
    return nc


def _get_nc(bl=BL):
    if bl not in _CACHE:
        _CACHE[bl] = build(bl)
    return _CACHE[bl]


def _prep_inputs(x, hs, We, be, Wd, bd, ws):
    """Host-side relayout/quantization for one core's shard."""
    import ml_dtypes

    f8 = ml_dtypes.float8_e4m3
    bf16 = ml_dtypes.bfloat16
    bl = x.shape[0]

    x8 = x.astype(f8)
    xlo = (16.0 * (x - x8.astype(np.float32))).astype(f8)
    # chunk view [bl, T, NH, P] -> [bl, P(h), chunk, T]
    X8 = np.ascontiguousarray(x8.reshape(bl, T, NH, P).transpose(0, 3, 2, 1))
    XLO = np.ascontiguousarray(xlo.reshape(bl, T, NH, P).transpose(0, 3, 2, 1))
    xp = np.empty((bl, P, NCALL, 2, T), dtype=f8)
    call = 0
    for c in range(K_PAIR // 2):
        xp[:, :, call, 0, :] = X8[:, :, 2 * c, :]
        xp[:, :, call, 1, :] = X8[:, :, 2 * c + 1, :]
        call += 1
    for ch in range(K_PAIR, NH):
        xp[:, :, call, 0, :] = X8[:, :, ch, :]
        xp[:, :, call, 1, :] = XLO[:, :, ch, :]
        call += 1

    xn = np.ascontiguousarray(
        x.astype(bf16).reshape(bl, NT, P, H).transpose(0, 2, 1, 3)
    )

    Wa = (We * 256.0).astype(f8)
    Wb = (We * 16.0).astype(f8)
    # [o, h] -> [ob, P(o), ch, P(h)] -> lhsT block [h, o] = .T per block
    WaT = Wa.reshape(NO, P, NH, P).transpose(2, 3, 0, 1)  # [ch, P(h), ob, P(o)]
    WbT = Wb.reshape(NO, P, NH, P).transpose(2, 3, 0, 1)
    wp = np.empty((NO, P, NCALL, 2, P), dtype=f8)
    for ob in range(NO):
        call = 0
        for c in range(K_PAIR // 2):
            wp[ob, :, call, 0, :] = WaT[2 * c, :, ob, :]
            wp[ob, :, call, 1, :] = WaT[2 * c + 1, :, ob, :]
            call += 1
        for ch in range(K_PAIR, NH):
            wp[ob, :, call, 0, :] = WaT[ch, :, ob, :]
            wp[ob, :, call, 1, :] = WbT[ch, :, ob, :]
            call += 1

    wdT = np.ascontiguousarray(Wd.T.astype(bf16).reshape(NH, P, H))
    hsT = np.ascontiguousarray(
        hs.astype(bf16).reshape(bl, NH, P).transpose(2, 1, 0).reshape(P, NH * bl)
    )
    ws_col = np.ascontiguousarray(ws.astype(bf16).reshape(NH, P).T)
    be_col = np.ascontiguousarray(be.reshape(NH, P).T.astype(np.float32))
    bd_col = np.ascontiguousarray(bd.reshape(NH, P).T.astype(np.float32))
    return {
        "x_pairs": xp, "x_nat": xn, "w_pairs": wp, "wdT": wdT,
        "hsT": hsT, "ws_col": ws_col, "be_col": be_col, "bd_col": bd_col,
    }


def kernel(**inputs):
    from concourse.bass_utils import run_bass_kernel_spmd

    x = np.ascontiguousarray(np.asarray(inputs["spatial_feats"], dtype=np.float32))
    hs = np.ascontiguousarray(np.asarray(inputs["hidden_state"], dtype=np.float32))
    We = np.asarray(inputs["W_enc"], dtype=np.float32)
    be = np.asarray(inputs["b_enc"], dtype=np.float32)
    Wd = np.asarray(inputs["W_dec"], dtype=np.float32)
    bd = np.asarray(inputs["b_dec"], dtype=np.float32)
    ws = np.asarray(inputs["w_score"], dtype=np.float32)

    nc = _get_nc()
    in_maps = []
    shared = None
    for i in range(NCORES):
        m = _prep_inputs(
            x[i * BL : (i + 1) * BL], hs[i * BL : (i + 1) * BL], We, be, Wd, bd, ws
        )
        if shared is None:
            shared = {k: m[k] for k in ("w_pairs", "wdT", "ws_col", "be_col", "bd_col")}
        else:
            m.update(shared)  # identical across cores; reuse arrays
        in_maps.append(m)
    res = run_bass_kernel_spmd(nc, in_maps, core_ids=list(range(NCORES)))
    global LAST_RESULT
    LAST_RESULT = res
    ctx = np.concatenate([res.results[i]["out_ctx"] for i in range(NCORES)], axis=0)
    w = np.concatenate([res.results[i]["out_w"] for i in range(NCORES)], axis=0)
    return (ctx, w)


# revision 16
# speedup vs baseline: 1.4578x; 1.0399x over previous
"""Bahdanau-attention kernel for Trainium2, data-parallel over 8 NeuronCores.

Per core (B_local=8, T=1024, H=1024), per batch b:
  encT[o,t] = sum_h W_enc[o,h] * x[t,h]   (PE: K_PAIR h-chunks as fp8 e4m3
      DoubleRow pairs -- 2 K-tiles of 128 per call at 2 cols/cycle -- the
      rest as exact bf16 calls. All W scaled by 256 -> tanh scale 2^-8.)
  energyT   = tanh(encT*2^-8 + (W_dec h + b_dec + b_enc)[o])  (ScalarE)
  scores[t] = sum_o w_score[o] * energyT[o,t]   (PE, bf16)
  weights   = softmax(scores)   (exp on ScalarE straight from PSUM,
      weight columnization via K=1 matmuls against a ones row)
  context   = sum_t weights[t] * x[t,:]         (PE, bf16 natural-layout x)
b_score dropped: softmax shift-invariance cancels it in both outputs.
All transposes are done host-side (xT fp8 pairs, W_encT fp8, W_decT/hsT
bf16, x natural bf16) -- no PE transposes in steady state.
"""

import sys

if "/opt/trn_rl_repo" not in sys.path:
    sys.path.insert(0, "/opt/trn_rl_repo")

import numpy as np

B, T, H = 64, 1024, 1024
NCORES = 8
BL = B // NCORES
P = 128
NT = T // P  # t chunks
NH = H // P  # h chunks
NO = H // P  # o blocks
F = 512      # PSUM bank free size (f32)
NS = T // F
K_PAIR = 6             # h-chunks contracted as fp8 DoubleRow pairs
NPAIR = K_PAIR // 2    # fp8 pair calls (2 chunks each)
NBF = NH - K_PAIR      # remaining h-chunks in exact bf16

_CACHE = {}
LAST_RESULT = None


def build(bl=BL):
    import concourse.tile as tile
    from concourse import bacc, mybir
    from concourse.masks import make_identity

    f32 = mybir.dt.float32
    f8 = mybir.dt.float8e4
    bf = mybir.dt.bfloat16
    AF = mybir.ActivationFunctionType
    DR = mybir.MatmulPerfMode.DoubleRow

    nc = bacc.Bacc("TRN2", target_bir_lowering=False, debug=False, num_devices=NCORES)
    xp_d = nc.declare_dram_parameter("x_pairs", [bl, P, NPAIR, 2, T], f8, isOutput=False)
    xb_d = nc.declare_dram_parameter("x_bf", [bl, P, NBF, T], bf, isOutput=False)
    xn_d = nc.declare_dram_parameter("x_nat", [bl, P, NT, H], bf, isOutput=False)
    wp_d = nc.declare_dram_parameter("w_pairs", [NO, P, NPAIR, 2, P], f8, isOutput=False)
    wb_d = nc.declare_dram_parameter("w_bf", [NBF, P, H], bf, isOutput=False)
    wdT_d = nc.declare_dram_parameter("wdT", [NH, P, H], bf, isOutput=False)
    hsT_d = nc.declare_dram_parameter("hsT", [P, NH * bl], bf, isOutput=False)
    ws_d = nc.declare_dram_parameter("ws_col", [P, NH], bf, isOutput=False)
    be_d = nc.declare_dram_parameter("be_col", [P, NH], f32, isOutput=False)
    bd_d = nc.declare_dram_parameter("bd_col", [P, NH], f32, isOutput=False)
    ctx_d = nc.declare_dram_parameter("out_ctx", [bl, H], f32, isOutput=True)
    wout_d = nc.declare_dram_parameter("out_w", [bl, T], f32, isOutput=True)

    with tile.TileContext(nc) as tc:
        with (
            tc.tile_pool(name="const", bufs=1) as const,
            tc.tile_pool(name="wenc", bufs=NO) as wenc_pool,
            tc.tile_pool(name="wdec", bufs=NH) as wdec_pool,
            tc.tile_pool(name="xp", bufs=2) as xp_pool,
            tc.tile_pool(name="xn", bufs=3) as xn_pool,
            tc.tile_pool(name="eT", bufs=6) as eT_pool,
            tc.tile_pool(name="rows", bufs=2) as rows,
            tc.tile_pool(name="small", bufs=2) as small,
            tc.tile_pool(name="mmps", bufs=4, space="PSUM") as mm_ps,
            tc.tile_pool(name="scps", bufs=2, space="PSUM") as sc_ps,
            tc.tile_pool(name="ctxps", bufs=2, space="PSUM") as ctx_ps,
        ):
            # ---- constants / weights DMA ----
            # sync queue carries everything batch-0 enc needs, in order;
            # gpsimd/scalar carry wdT (dec path) then batch-0 xn.
            hsT = const.tile([P, NH * bl], bf, tag="hsT")
            nc.sync.dma_start(hsT[:], hsT_d[:, :])
            ws_sb = const.tile([P, NH], bf, tag="ws")
            nc.sync.dma_start(ws_sb[:], ws_d[:, :])
            be_sb = const.tile([P, NH], f32, tag="be")
            nc.sync.dma_start(be_sb[:], be_d[:, :])
            bd_sb = const.tile([P, NH], f32, tag="bd")
            nc.sync.dma_start(bd_sb[:], bd_d[:, :])
            bsum = const.tile([P, NH], f32, tag="bsum")
            nc.vector.tensor_add(bsum[:], be_sb[:], bd_sb[:])

            ident_f = const.tile([P, P], f32, tag="identf")
            make_identity(nc, ident_f[:])
            ones_f = const.tile([1, NT], f32, tag="onesf")
            nc.vector.memset(ones_f[:], 1.0)
            ones8 = const.tile([1, NT], bf, tag="ones8")
            nc.vector.tensor_copy(ones8[:], ones_f[:])

            wdT = []
            for c in range(NH):
                t = wdec_pool.tile([P, H], bf, tag="wdT")
                eng = nc.gpsimd if c % 2 == 0 else nc.scalar
                eng.dma_start(t[:], wdT_d[c, :, :])
                wdT.append(t)

            wT = []
            for ob in range(NO):
                t = wenc_pool.tile([P, NPAIR, 2, P], f8, tag="wenc")
                nc.sync.dma_start(t[:], wp_d[ob, :, :, :, :])
                wT.append(t)
            wB = []
            for j in range(NBF):
                t = wenc_pool.tile([P, H], bf, tag="wbf")
                nc.sync.dma_start(t[:], wb_d[j, :, :])
                wB.append(t)

            # ---- per-batch x DMA ----
            def dma_x(it):
                xp_t = xp_pool.tile([P, NPAIR, 2, T], f8, tag="xp")
                nc.sync.dma_start(xp_t[:], xp_d[it, :, :, :, :])
                xb_t = xp_pool.tile([P, NBF, T], bf, tag="xb")
                nc.sync.dma_start(xb_t[:], xb_d[it, :, :, :])
                xn_t = xn_pool.tile([P, NT, H], bf, tag="xn")
                nc.gpsimd.dma_start(xn_t[:, 0:4, :], xn_d[it, :, 0:4, :])
                nc.scalar.dma_start(xn_t[:, 4:8, :], xn_d[it, :, 4:8, :])
                return (xp_t, xb_t), xn_t

            xp_cur, xn_cur = dma_x(0)

            # ---- dec bias: dec[b,o] = sum_h hs[b,h] Wd[o,h]; then transpose ----
            # chunk-major order so each wdT chunk is consumed as its DMA lands
            dec_sb = const.tile([bl, H], f32, tag="decsb")
            psd = [mm_ps.tile([P, F], f32, tag="mmps", name=f"psd{h}")
                   for h in range(NS)]
            for c in range(NH):
                for half in range(NS):
                    nc.tensor.matmul(
                        psd[half][0:bl, :],
                        hsT[:, c * bl : (c + 1) * bl],
                        wdT[c][:, half * F : (half + 1) * F],
                        start=(c == 0),
                        stop=(c == NH - 1),
                    )
            for half in range(NS):
                nc.vector.tensor_copy(
                    dec_sb[:, half * F : (half + 1) * F], psd[half][0:bl, :]
                )
            bias_all = const.tile([P, NO * bl], f32, tag="bias_all")
            for ob in range(NO):
                psT = mm_ps.tile([P, F], f32, tag="mmps")
                nc.tensor.transpose(
                    psT[:, 0:bl], dec_sb[:, ob * P : (ob + 1) * P], ident_f[0:bl, 0:bl]
                )
                nc.vector.tensor_scalar_add(
                    bias_all[:, ob * bl : (ob + 1) * bl], psT[:, 0:bl],
                    bsum[:, ob : ob + 1],
                )

            # ---- helpers ----
            def enc_ob(it, ob, x_cur):
                """enc+tanh for one o-block; returns (eT_half0, eT_half1)."""
                xp_t, xb_t = x_cur
                psE = [mm_ps.tile([P, F], f32, tag="mmps", name=f"psE{ob}_{h}")
                       for h in range(NS)]
                for c in range(NPAIR):
                    for half in range(NS):
                        nc.tensor.matmul(
                            psE[half][:],
                            wT[ob][:, c, :, :],
                            xp_t[:, c, :, half * F : (half + 1) * F],
                            start=(c == 0),
                            stop=False,
                            perf_mode=DR,
                        )
                for j in range(NBF):
                    for half in range(NS):
                        nc.tensor.matmul(
                            psE[half][:],
                            wB[j][:, ob * P : (ob + 1) * P],
                            xb_t[:, j, half * F : (half + 1) * F],
                            start=False,
                            stop=(j == NBF - 1),
                        )
                es = []
                for half in range(NS):
                    e = eT_pool.tile([P, F], bf, tag="eT")
                    nc.scalar.activation(
                        e[:], psE[half][:], AF.Tanh,
                        bias=bias_all[:, ob * bl + it : ob * bl + it + 1],
                        scale=1.0 / 256.0,
                    )
                    es.append(e)
                return es

            def score_ob(ob, ets, pss):
                for half in range(NS):
                    nc.tensor.matmul(
                        pss[half][:],
                        ws_sb[:, ob : ob + 1],
                        ets[ob][half][:],
                        start=(ob == 0),
                        stop=(ob == NO - 1),
                    )

            # ---- main pipeline ----
            prev = None
            for it in range(bl + 1):
                if 0 < it < bl:
                    xp_cur, xn_cur = dma_x(it)

                if it < bl:
                    # o-block 0 first: gives PE work while prev softmax drains
                    ets = [enc_ob(it, 0, xp_cur)]

                if it >= 1:
                    st = prev
                    # columnize exp-weights: uT[:, c] = u[c*128:(c+1)*128]
                    psw = mm_ps.tile([P, NT * NT], f32, tag="mmps")
                    for c in range(NT):
                        nc.tensor.matmul(
                            psw[:, c * NT : (c + 1) * NT],
                            st["u_rowr"][0:1, c * P : (c + 1) * P],
                            ones8[:],
                            start=(c == 0),
                            stop=(c == NT - 1),
                        )
                    uT = small.tile([P, NT], bf, tag="uT")
                    nc.vector.tensor_copy(uT[:], psw[:, 0 : NT * NT : NT])
                    ctx_row = rows.tile([1, H], f32, tag="ctxrow")
                    for half in range(NS):
                        pc = ctx_ps.tile([1, F], f32, tag="ctxps")
                        for c in range(NT):
                            nc.tensor.matmul(
                                pc[:],
                                uT[:, c : c + 1],
                                st["xn"][:, c, half * F : (half + 1) * F],
                                start=(c == 0),
                                stop=(c == NT - 1),
                            )
                        nc.vector.tensor_scalar_mul(
                            ctx_row[0:1, half * F : (half + 1) * F],
                            pc[0:1, :],
                            st["rz"][0:1, 0:1],
                        )
                    nc.sync.dma_start(ctx_d[it - 1 : it, :], ctx_row[:])
                    w_row = rows.tile([1, T], f32, tag="wrow")
                    nc.vector.tensor_scalar_mul(
                        w_row[:], st["u_row"][:], st["rz"][0:1, 0:1]
                    )
                    nc.sync.dma_start(wout_d[it - 1 : it, :], w_row[:])

                if it < bl:
                    pss = [sc_ps.tile([1, F], f32, tag="scps", name=f"pss{h}")
                           for h in range(NS)]
                    for ob in range(1, NO):
                        ets.append(enc_ob(it, ob, xp_cur))
                        score_ob(ob - 1, ets, pss)
                    score_ob(NO - 1, ets, pss)
                    # softmax pieces: exp straight from PSUM
                    u_rowr = rows.tile([1, T], bf, tag="urowr")
                    u_row = rows.tile([1, T], f32, tag="urow")
                    ssum = small.tile([1, NS], f32, tag="ssum")
                    for half in range(NS):
                        nc.scalar.activation(
                            u_rowr[0:1, half * F : (half + 1) * F],
                            pss[half][0:1, :], AF.Exp, bias=0.0, scale=1.0,
                        )
                    for half in range(NS):
                        nc.scalar.activation(
                            u_row[0:1, half * F : (half + 1) * F],
                            pss[half][0:1, :], AF.Exp, bias=0.0, scale=1.0,
                            accum_out=ssum[0:1, half : half + 1],
                        )
                    stot = small.tile([1, 1], f32, tag="stot")
                    nc.vector.tensor_add(stot[:], ssum[0:1, 0:1], ssum[0:1, 1:2])
                    rz = small.tile([1, 1], f32, tag="rz")
                    nc.vector.reciprocal(rz[:], stot[:])
                    prev = {"u_rowr": u_rowr, "u_row": u_row, "rz": rz, "xn": xn_cur}

    nc.compile()
    return nc


def _get_nc(bl=BL):
    if bl not in _CACHE:
        _CACHE[bl] = build(bl)
    return _CACHE[bl]


def _prep_inputs(x, hs, We, be, Wd, bd, ws):
    """Host-side relayout/quantization for one core's shard."""
    import ml_dtypes

    f8 = ml_dtypes.float8_e4m3
    bf16 = ml_dtypes.bfloat16
    bl = x.shape[0]

    # x^T chunk view: [bl, T, NH, P] -> [bl, P(h), chunk, T]
    X8 = np.ascontiguousarray(
        x.astype(f8).reshape(bl, T, NH, P).transpose(0, 3, 2, 1)
    )
    X16 = np.ascontiguousarray(
        x.astype(bf16).reshape(bl, T, NH, P).transpose(0, 3, 2, 1)
    )
    xp = np.empty((bl, P, NPAIR, 2, T), dtype=f8)
    for c in range(NPAIR):
        xp[:, :, c, 0, :] = X8[:, :, 2 * c, :]
        xp[:, :, c, 1, :] = X8[:, :, 2 * c + 1, :]
    xb = np.ascontiguousarray(X16[:, :, K_PAIR:, :])

    xn = np.ascontiguousarray(
        x.astype(bf16).reshape(bl, NT, P, H).transpose(0, 2, 1, 3)
    )

    Wa = (We * 256.0).astype(f8)
    # [o, h] -> [ch, P(h), ob, P(o)]: lhsT block layout
    WaT = Wa.reshape(NO, P, NH, P).transpose(2, 3, 0, 1)
    wp = np.empty((NO, P, NPAIR, 2, P), dtype=f8)
    for ob in range(NO):
        for c in range(NPAIR):
            wp[ob, :, c, 0, :] = WaT[2 * c, :, ob, :]
            wp[ob, :, c, 1, :] = WaT[2 * c + 1, :, ob, :]
    # bf16 chunks, scaled by 256 to share the fp8 PSUM scale
    W16T = (We * 256.0).astype(bf16).reshape(NO, P, NH, P).transpose(2, 3, 0, 1)
    wb = np.ascontiguousarray(W16T[K_PAIR:]).reshape(NBF, P, H)

    wdT = np.ascontiguousarray(Wd.T.astype(bf16).reshape(NH, P, H))
    hsT = np.ascontiguousarray(
        hs.astype(bf16).reshape(bl, NH, P).transpose(2, 1, 0).reshape(P, NH * bl)
    )
    ws_col = np.ascontiguousarray(ws.astype(bf16).reshape(NH, P).T)
    be_col = np.ascontiguousarray(be.reshape(NH, P).T.astype(np.float32))
    bd_col = np.ascontiguousarray(bd.reshape(NH, P).T.astype(np.float32))
    return {
        "x_pairs": xp, "x_bf": xb, "x_nat": xn, "w_pairs": wp, "w_bf": wb,
        "wdT": wdT, "hsT": hsT, "ws_col": ws_col, "be_col": be_col,
        "bd_col": bd_col,
    }


def kernel(**inputs):
    from concourse.bass_utils import run_bass_kernel_spmd

    x = np.ascontiguousarray(np.asarray(inputs["spatial_feats"], dtype=np.float32))
    hs = np.ascontiguousarray(np.asarray(inputs["hidden_state"], dtype=np.float32))
    We = np.asarray(inputs["W_enc"], dtype=np.float32)
    be = np.asarray(inputs["b_enc"], dtype=np.float32)
    Wd = np.asarray(inputs["W_dec"], dtype=np.float32)
    bd = np.asarray(inputs["b_dec"], dtype=np.float32)
    ws = np.asarray(inputs["w_score"], dtype=np.float32)

    nc = _get_nc()
    in_maps = []
    shared = None
    for i in range(NCORES):
        m = _prep_inputs(
            x[i * BL : (i + 1) * BL], hs[i * BL : (i + 1) * BL], We, be, Wd, bd, ws
        )
        if shared is None:
            shared = {k: m[k] for k in
                      ("w_pairs", "w_bf", "wdT", "ws_col", "be_col", "bd_col")}
        else:
            m.update(shared)  # identical across cores; reuse arrays
        in_maps.append(m)
    res = run_bass_kernel_spmd(nc, in_maps, core_ids=list(range(NCORES)))
    global LAST_RESULT
    LAST_RESULT = res
    ctx = np.concatenate([res.results[i]["out_ctx"] for i in range(NCORES)], axis=0)
    w = np.concatenate([res.results[i]["out_w"] for i in range(NCORES)], axis=0)
    return (ctx, w)


# revision 23
# speedup vs baseline: 1.5937x; 1.0932x over previous
"""Bahdanau-attention kernel for Trainium2, data-parallel over 8 NeuronCores.

Per core (B_local=8, T=1024, H=1024), per batch b:
  encT[o,t] = sum_h W_enc[o,h] * x[t,h]   (PE: K_PAIR h-chunks as fp8 e4m3
      DoubleRow pairs -- 2 K-tiles of 128 per call at 2 cols/cycle -- the
      rest as exact bf16 calls. All W scaled by 256 -> tanh scale 2^-8.)
  energyT   = tanh(encT*2^-8 + (W_dec h + b_dec + b_enc)[o])  (ScalarE)
  scores[t] = sum_o w_score[o] * energyT[o,t]   (PE, bf16)
  weights   = softmax(scores)   (exp on ScalarE straight from PSUM,
      weight columnization via K=1 matmuls against a ones row)
  context   = sum_t weights[t] * x[t,:]         (PE, bf16 natural-layout x)
b_score dropped: softmax shift-invariance cancels it in both outputs.
All transposes are done host-side (xT fp8 pairs, W_encT fp8, W_decT/hsT
bf16, x natural bf16) -- no PE transposes in steady state.
"""

import sys

if "/opt/trn_rl_repo" not in sys.path:
    sys.path.insert(0, "/opt/trn_rl_repo")

import numpy as np

B, T, H = 64, 1024, 1024
NCORES = 8
BL = B // NCORES
P = 128
NT = T // P  # t chunks
NH = H // P  # h chunks
NO = H // P  # o blocks
F = 512      # PSUM bank free size (f32)
NS = T // F
K_PAIR = 6             # h-chunks contracted as fp8 DoubleRow pairs
NPAIR = K_PAIR // 2    # fp8 pair calls (2 chunks each)
NBF = NH - K_PAIR      # remaining h-chunks in exact bf16

_CACHE = {}
LAST_RESULT = None


def build(bl=BL):
    import concourse.tile as tile
    from concourse import bacc, mybir
    from concourse.masks import make_identity

    f32 = mybir.dt.float32
    f8 = mybir.dt.float8e4
    bf = mybir.dt.bfloat16
    AF = mybir.ActivationFunctionType
    DR = mybir.MatmulPerfMode.DoubleRow

    nc = bacc.Bacc("TRN2", target_bir_lowering=False, debug=False, num_devices=NCORES)
    xp_d = nc.declare_dram_parameter("x_pairs", [bl, P, NPAIR, 2, T], f8, isOutput=False)
    xb_d = nc.declare_dram_parameter("x_bf", [bl, P, NBF, T], bf, isOutput=False)
    xn_d = nc.declare_dram_parameter("x_nat", [bl, P, NT, H], bf, isOutput=False)
    wp_d = nc.declare_dram_parameter("w_pairs", [P, NO, NPAIR, 2, P], f8, isOutput=False)
    wb_d = nc.declare_dram_parameter("w_bf", [P, NBF, H], bf, isOutput=False)
    wdT_d = nc.declare_dram_parameter("wdT", [P, NH, H], bf, isOutput=False)
    hsT_d = nc.declare_dram_parameter("hsT", [P, NH * bl], bf, isOutput=False)
    ws_d = nc.declare_dram_parameter("ws_col", [P, NH], bf, isOutput=False)
    be_d = nc.declare_dram_parameter("be_col", [P, NH], f32, isOutput=False)
    bd_d = nc.declare_dram_parameter("bd_col", [P, NH], f32, isOutput=False)
    ctx_d = nc.declare_dram_parameter("out_ctx", [bl, H], f32, isOutput=True)
    wout_d = nc.declare_dram_parameter("out_w", [bl, T], f32, isOutput=True)

    with tile.TileContext(nc) as tc:
        with (
            tc.tile_pool(name="const", bufs=1) as const,
            tc.tile_pool(name="xp", bufs=2) as xp_pool,
            tc.tile_pool(name="xn", bufs=3) as xn_pool,
            tc.tile_pool(name="eT", bufs=6) as eT_pool,
            tc.tile_pool(name="rows", bufs=2) as rows,
            tc.tile_pool(name="small", bufs=2) as small,
            tc.tile_pool(name="mmps", bufs=4, space="PSUM") as mm_ps,
            tc.tile_pool(name="scps", bufs=2, space="PSUM") as sc_ps,
            tc.tile_pool(name="ctxps", bufs=2, space="PSUM") as ctx_ps,
        ):
            # ---- constants built first (gpsimd/vector heads, before DMA issues) ----
            ident_f = const.tile([P, P], f32, tag="identf")
            make_identity(nc, ident_f[:])
            ones_f = const.tile([1, NT], f32, tag="onesf")
            nc.vector.memset(ones_f[:], 1.0)
            ones8 = const.tile([1, NT], bf, tag="ones8")
            nc.vector.tensor_copy(ones8[:], ones_f[:])

            # ---- weights/batch-0 DMA; xn[0] deferred out of this window ----
            # sync: operands for batch-0 enc, pairs first; gpsimd/scalar: wdT.
            hsT = const.tile([P, NH * bl], bf, tag="hsT")
            nc.sync.dma_start(hsT[:], hsT_d[:, :])
            wTt = const.tile([P, NO, NPAIR, 2, P], f8, tag="wenc")
            nc.sync.dma_start(wTt[:], wp_d[:, :, :, :, :])
            xp0 = xp_pool.tile([P, NPAIR, 2, T], f8, tag="xp")
            nc.sync.dma_start(xp0[:], xp_d[0, :, :, :, :])
            wBt = const.tile([P, NBF, H], bf, tag="wbf")
            nc.sync.dma_start(wBt[:], wb_d[:, :, :])
            xb0 = xp_pool.tile([P, NBF, T], bf, tag="xb")
            nc.sync.dma_start(xb0[:], xb_d[0, :, :, :])

            wdTt = const.tile([P, NH, H], bf, tag="wdT")
            nc.gpsimd.dma_start(wdTt[:, 0:4, :], wdT_d[:, 0:4, :])
            nc.scalar.dma_start(wdTt[:, 4:8, :], wdT_d[:, 4:8, :])
            ws_sb = const.tile([P, NH], bf, tag="ws")
            nc.gpsimd.dma_start(ws_sb[:], ws_d[:, :])
            be_sb = const.tile([P, NH], f32, tag="be")
            nc.gpsimd.dma_start(be_sb[:], be_d[:, :])
            bd_sb = const.tile([P, NH], f32, tag="bd")
            nc.gpsimd.dma_start(bd_sb[:], bd_d[:, :])
            bsum = const.tile([P, NH], f32, tag="bsum")
            nc.vector.tensor_add(bsum[:], be_sb[:], bd_sb[:])

            # ---- PE warmup: keep the array busy so HAM ramps during DMA wait
            warm_ps = mm_ps.tile([P, P], f32, tag="mmps", name="warm")
            for _ in range(16):
                nc.tensor.transpose(warm_ps[:], ident_f[:], ident_f[:])

            # ---- per-batch x DMA ----
            def dma_x(it):
                xp_t = xp_pool.tile([P, NPAIR, 2, T], f8, tag="xp")
                nc.sync.dma_start(xp_t[:], xp_d[it, :, :, :, :])
                xb_t = xp_pool.tile([P, NBF, T], bf, tag="xb")
                nc.sync.dma_start(xb_t[:], xb_d[it, :, :, :])
                xn_t = xn_pool.tile([P, NT, H], bf, tag="xn")
                nc.gpsimd.dma_start(xn_t[:, 0:4, :], xn_d[it, :, 0:4, :])
                nc.gpsimd.dma_start(xn_t[:, 4:8, :], xn_d[it, :, 4:8, :])
                return (xp_t, xb_t), xn_t

            xp_cur = (xp0, xb0)
            xn0 = xn_pool.tile([P, NT, H], bf, tag="xn")
            nc.gpsimd.dma_start(xn0[:, 0:4, :], xn_d[0, :, 0:4, :])
            nc.gpsimd.dma_start(xn0[:, 4:8, :], xn_d[0, :, 4:8, :])
            xn_cur = xn0

            # ---- dec bias: dec[b,o] = sum_h hs[b,h] Wd[o,h]; then transpose ----
            # chunk-major order so each wdT chunk is consumed as its DMA lands
            dec_sb = const.tile([bl, H], f32, tag="decsb")
            psd = [mm_ps.tile([P, F], f32, tag="mmps", name=f"psd{h}")
                   for h in range(NS)]
            for c in range(NH):
                for half in range(NS):
                    nc.tensor.matmul(
                        psd[half][0:bl, :],
                        hsT[:, c * bl : (c + 1) * bl],
                        wdTt[:, c, half * F : (half + 1) * F],
                        start=(c == 0),
                        stop=(c == NH - 1),
                    )
            for half in range(NS):
                nc.vector.tensor_copy(
                    dec_sb[:, half * F : (half + 1) * F], psd[half][0:bl, :]
                )
            bias_all = const.tile([P, NO * bl], f32, tag="bias_all")
            for ob in range(NO):
                psT = mm_ps.tile([P, F], f32, tag="mmps")
                nc.tensor.transpose(
                    psT[:, 0:bl], dec_sb[:, ob * P : (ob + 1) * P], ident_f[0:bl, 0:bl]
                )
                nc.vector.tensor_scalar_add(
                    bias_all[:, ob * bl : (ob + 1) * bl], psT[:, 0:bl],
                    bsum[:, ob : ob + 1],
                )

            # ---- helpers ----
            def enc_ob(it, ob, x_cur):
                """enc+tanh for one o-block; returns (eT_half0, eT_half1)."""
                xp_t, xb_t = x_cur
                psE = [mm_ps.tile([P, F], f32, tag="mmps", name=f"psE{ob}_{h}")
                       for h in range(NS)]
                for c in range(NPAIR):
                    for half in range(NS):
                        nc.tensor.matmul(
                            psE[half][:],
                            wTt[:, ob, c, :, :],
                            xp_t[:, c, :, half * F : (half + 1) * F],
                            start=(c == 0),
                            stop=False,
                            perf_mode=DR,
                        )
                for j in range(NBF):
                    for half in range(NS):
                        nc.tensor.matmul(
                            psE[half][:],
                            wBt[:, j, ob * P : (ob + 1) * P],
                            xb_t[:, j, half * F : (half + 1) * F],
                            start=False,
                            stop=(j == NBF - 1),
                        )
                es = []
                for half in range(NS):
                    e = eT_pool.tile([P, F], bf, tag="eT")
                    nc.scalar.activation(
                        e[:], psE[half][:], AF.Tanh,
                        bias=bias_all[:, ob * bl + it : ob * bl + it + 1],
                        scale=1.0 / 256.0,
                    )
                    es.append(e)
                return es

            def score_ob(ob, ets, pss):
                for half in range(NS):
                    nc.tensor.matmul(
                        pss[half][:],
                        ws_sb[:, ob : ob + 1],
                        ets[ob][half][:],
                        start=(ob == 0),
                        stop=(ob == NO - 1),
                    )

            # ---- main pipeline ----
            prev = None
            for it in range(bl + 1):
                if 0 < it < bl:
                    xp_cur, xn_cur = dma_x(it)

                if it < bl:
                    # o-block 0 first: gives PE work while prev softmax drains
                    ets = [enc_ob(it, 0, xp_cur)]

                if it >= 1:
                    st = prev
                    # columnize exp-weights: uT[:, c] = u[c*128:(c+1)*128]
                    psw = mm_ps.tile([P, NT * NT], f32, tag="mmps")
                    for c in range(NT):
                        nc.tensor.matmul(
                            psw[:, c * NT : (c + 1) * NT],
                            st["u_rowr"][0:1, c * P : (c + 1) * P],
                            ones8[:],
                            start=(c == 0),
                            stop=(c == NT - 1),
                        )
                    uT = small.tile([P, NT], bf, tag="uT")
                    nc.vector.tensor_copy(uT[:], psw[:, 0 : NT * NT : NT])
                    ctx_row = rows.tile([1, H], f32, tag="ctxrow")
                    for half in range(NS):
                        pc = ctx_ps.tile([1, F], f32, tag="ctxps")
                        for c in range(NT):
                            nc.tensor.matmul(
                                pc[:],
                                uT[:, c : c + 1],
                                st["xn"][:, c, half * F : (half + 1) * F],
                                start=(c == 0),
                                stop=(c == NT - 1),
                            )
                        nc.vector.tensor_scalar_mul(
                            ctx_row[0:1, half * F : (half + 1) * F],
                            pc[0:1, :],
                            st["rz"][0:1, 0:1],
                        )
                    nc.sync.dma_start(ctx_d[it - 1 : it, :], ctx_row[:])
                    w_row = rows.tile([1, T], f32, tag="wrow")
                    nc.vector.tensor_scalar_mul(
                        w_row[:], st["u_row"][:], st["rz"][0:1, 0:1]
                    )
                    nc.sync.dma_start(wout_d[it - 1 : it, :], w_row[:])

                if it < bl:
                    pss = [sc_ps.tile([1, F], f32, tag="scps", name=f"pss{h}")
                           for h in range(NS)]
                    for ob in range(1, NO):
                        ets.append(enc_ob(it, ob, xp_cur))
                        score_ob(ob - 1, ets, pss)
                    score_ob(NO - 1, ets, pss)
                    # softmax pieces: exp straight from PSUM
                    u_rowr = rows.tile([1, T], bf, tag="urowr")
                    u_row = rows.tile([1, T], f32, tag="urow")
                    ssum = small.tile([1, NS], f32, tag="ssum")
                    for half in range(NS):
                        nc.scalar.activation(
                            u_rowr[0:1, half * F : (half + 1) * F],
                            pss[half][0:1, :], AF.Exp, bias=0.0, scale=1.0,
                        )
                    for half in range(NS):
                        nc.scalar.activation(
                            u_row[0:1, half * F : (half + 1) * F],
                            pss[half][0:1, :], AF.Exp, bias=0.0, scale=1.0,
                            accum_out=ssum[0:1, half : half + 1],
                        )
                    stot = small.tile([1, 1], f32, tag="stot")
                    nc.vector.tensor_add(stot[:], ssum[0:1, 0:1], ssum[0:1, 1:2])
                    rz = small.tile([1, 1], f32, tag="rz")
                    nc.vector.reciprocal(rz[:], stot[:])
                    prev = {"u_rowr": u_rowr, "u_row": u_row, "rz": rz, "xn": xn_cur}

    nc.compile()
    return nc


def _get_nc(bl=BL):
    if bl not in _CACHE:
        _CACHE[bl] = build(bl)
    return _CACHE[bl]


def _prep_inputs(x, hs, We, be, Wd, bd, ws):
    """Host-side relayout/quantization for one core's shard."""
    import ml_dtypes

    f8 = ml_dtypes.float8_e4m3
    bf16 = ml_dtypes.bfloat16
    bl = x.shape[0]

    # x^T chunk view: [bl, T, NH, P] -> [bl, P(h), chunk, T]
    X8 = np.ascontiguousarray(
        x.astype(f8).reshape(bl, T, NH, P).transpose(0, 3, 2, 1)
    )
    X16 = np.ascontiguousarray(
        x.astype(bf16).reshape(bl, T, NH, P).transpose(0, 3, 2, 1)
    )
    xp = np.empty((bl, P, NPAIR, 2, T), dtype=f8)
    for c in range(NPAIR):
        xp[:, :, c, 0, :] = X8[:, :, 2 * c, :]
        xp[:, :, c, 1, :] = X8[:, :, 2 * c + 1, :]
    xb = np.ascontiguousarray(X16[:, :, K_PAIR:, :])

    xn = np.ascontiguousarray(
        x.astype(bf16).reshape(bl, NT, P, H).transpose(0, 2, 1, 3)
    )

    Wa = (We * 256.0).astype(f8)
    # [o, h] -> [ch, P(h), ob, P(o)]: lhsT block layout
    WaT = Wa.reshape(NO, P, NH, P).transpose(2, 3, 0, 1)
    wp = np.empty((P, NO, NPAIR, 2, P), dtype=f8)
    for ob in range(NO):
        for c in range(NPAIR):
            wp[:, ob, c, 0, :] = WaT[2 * c, :, ob, :]
            wp[:, ob, c, 1, :] = WaT[2 * c + 1, :, ob, :]
    # bf16 chunks, scaled by 256 to share the fp8 PSUM scale
    W16T = (We * 256.0).astype(bf16).reshape(NO, P, NH, P).transpose(3, 2, 0, 1)
    wb = np.ascontiguousarray(W16T[:, K_PAIR:]).reshape(P, NBF, H)

    wdT = np.ascontiguousarray(
        Wd.T.astype(bf16).reshape(NH, P, H).transpose(1, 0, 2)
    )
    hsT = np.ascontiguousarray(
        hs.astype(bf16).reshape(bl, NH, P).transpose(2, 1, 0).reshape(P, NH * bl)
    )
    ws_col = np.ascontiguousarray(ws.astype(bf16).reshape(NH, P).T)
    be_col = np.ascontiguousarray(be.reshape(NH, P).T.astype(np.float32))
    bd_col = np.ascontiguousarray(bd.reshape(NH, P).T.astype(np.float32))
    return {
        "x_pairs": xp, "x_bf": xb, "x_nat": xn, "w_pairs": wp, "w_bf": wb,
        "wdT": wdT, "hsT": hsT, "ws_col": ws_col, "be_col": be_col,
        "bd_col": bd_col,
    }


def kernel(**inputs):
    from concourse.bass_utils import run_bass_kernel_spmd

    x = np.ascontiguousarray(np.asarray(inputs["spatial_feats"], dtype=np.float32))
    hs = np.ascontiguousarray(np.asarray(inputs["hidden_state"], dtype=np.float32))
    We = np.asarray(inputs["W_enc"], dtype=np.float32)
    be = np.asarray(inputs["b_enc"], dtype=np.float32)
    Wd = np.asarray(inputs["W_dec"], dtype=np.float32)
    bd = np.asarray(inputs["b_dec"], dtype=np.float32)
    ws = np.asarray(inputs["w_score"], dtype=np.float32)

    nc = _get_nc()
    in_maps = []
    shared = None
    for i in range(NCORES):
        m = _prep_inputs(
            x[i * BL : (i + 1) * BL], hs[i * BL : (i + 1) * BL], We, be, Wd, bd, ws
        )
        if shared is None:
            shared = {k: m[k] for k in
                      ("w_pairs", "w_bf", "wdT", "ws_col", "be_col", "bd_col")}
        else:
            m.update(shared)  # identical across cores; reuse arrays
        in_maps.append(m)
    res = run_bass_kernel_spmd(nc, in_maps, core_ids=list(range(NCORES)))
    global LAST_RESULT
    LAST_RESULT = res
    ctx = np.concatenate([res.results[i]["out_ctx"] for i in range(NCORES)], axis=0)
    w = np.concatenate([res.results[i]["out_w"] for i in range(NCORES)], axis=0)
    return (ctx, w)
